# revision 21
# baseline (speedup 1.0000x reference)
"""Trainium2 Bass kernel for an HGT-style heterogeneous-graph-transformer layer.

Contract: kernel(**inputs) takes the FULL unsharded inputs (numpy arrays, keys
as in setup_inputs) and returns the FULL [50000, 128] float32 output.

Strategy (8 NeuronCores, SPMD):
  - Destination-node sharding. Host permutes nodes: each core owns an equal
    per-type slice (static type boundaries), nodes degree-balanced across cores
    and across 128-node blocks by snake-dealing.
  - Per core, per 128-dst-node block, incoming edges are grouped and padded to
    relation-pure tiles of 128 edges (fixed tile counts T_r per relation, so
    one static program serves all cores).
  - Stage 1 (node phase): every core computes the full fp16 K|V table
    (kv_table[n] = [x@Wk[type], x@Wv[type]]) replicated, plus qrel tables for
    its own nodes: qrel_r = (x@Wq[type] + bq) @ Bq_r where Bq_r is the
    block-diagonal per-head rel_att^T with pri/sqrt(dk) baked in.
  - Stage 2 (edge phase): indirect-DMA gathers of kv rows (by src) and qrel
    rows (by relation*dst); att = per-head dot via DVE mult + grouped reduce;
    exp on ACT; segment softmax denominator and per-relation partial
    aggregation via one-hot matmuls on PE; normalization applied at the node
    level after aggregation (softmax is linear in the numerator);
    aggrT = sum_r Bmsg_r^T-matmul of normalized partials.
  - Stage 3: gelu -> per-type update GEMM -> residual -> layernorm -> gamma/beta.
"""

import math

import numpy as np
import ml_dtypes

import concourse.bacc as bacc
import concourse.bass as bass
import concourse.mybir as mybir
import concourse.tile as tile
from concourse.bass import IndirectOffsetOnAxis
from concourse.bass_utils import run_bass_kernel_spmd

F32 = mybir.dt.float32
F16 = mybir.dt.float16
BF16 = mybir.dt.bfloat16
I32 = mybir.dt.int32

NCORES = 8
N, E, HID, H, DK, KN, KR = 50000, 500000, 128, 8, 16, 3, 3
SQRT_DK = np.sqrt(np.float32(DK)).astype(np.float32)
LN_EPS = 1e-5
P = 128
DUMMY_DSTL = 500.0  # one-hot never matches -> padded edges contribute nothing


# ----------------------------------------------------------------------------
# Host-side preprocessing: permutation, sharding, edge tiling, weight packing
# ----------------------------------------------------------------------------

def _snake_deal(items, nbins):
    """Deal items (sorted desc by weight) across nbins, snaking."""
    bins = [[] for _ in range(nbins)]
    for i, it in enumerate(items):
        rnd, j = divmod(i, nbins)
        bins[j if rnd % 2 == 0 else nbins - 1 - j].append(it)
    return bins


def _prep(inputs):
    x = np.asarray(inputs["x"], dtype=np.float32)
    node_type = np.asarray(inputs["node_type"]).astype(np.int64)
    edge_src = np.asarray(inputs["edge_src"]).astype(np.int64)
    edge_dst = np.asarray(inputs["edge_dst"]).astype(np.int64)
    edge_type = np.asarray(inputs["edge_type"]).astype(np.int64)

    deg = np.bincount(edge_dst, minlength=N)

    # --- node permutation: per-type, degree-balanced across cores ---
    K_t = [int(math.ceil((node_type == t).sum() / NCORES)) for t in range(KN)]
    Ncap = ((sum(K_t) + P - 1) // P) * P
    NBLK = Ncap // P
    cum_K = np.concatenate([[0], np.cumsum(K_t)])

    perm = np.full(NCORES * Ncap, -1, dtype=np.int64)  # new slot -> old id
    for t in range(KN):
        nodes_t = np.where(node_type == t)[0]
        nodes_t = nodes_t[np.argsort(-deg[nodes_t], kind="stable")]
        per_core = _snake_deal(list(nodes_t), NCORES)
        for c in range(NCORES):
            seg = np.array(per_core[c], dtype=np.int64)
            L = len(seg)
            if L == 0:
                continue
            # interleave by degree across the segment's 128-strides so
            # consecutive-128 blocks get balanced degree sums
            nstr = max(1, (L + P - 1) // P)
            order = sorted(range(L), key=lambda i: (i % nstr, i // nstr))
            seg = seg[np.array(order)]
            base = c * Ncap + cum_K[t]
            perm[base: base + L] = seg

    valid = perm >= 0
    new_of_old = np.full(N, -1, dtype=np.int64)
    new_of_old[perm[valid]] = np.where(valid)[0]
    assert (new_of_old >= 0).all()

    src_new = new_of_old[edge_src]
    dst_new = new_of_old[edge_dst]
    dst_core = dst_new // Ncap
    dst_slot = dst_new % Ncap

    # --- edge tiling: (core, block, relation)-pure tiles, padded ---
    order = np.lexsort((edge_type, dst_slot // P, dst_core))
    eo = order  # edges sorted by (core, block, r)
    counts = np.zeros((NCORES, NBLK, KR), dtype=np.int64)
    np.add.at(counts, (dst_core, dst_slot // P, edge_type), 1)
    T_r = [int(math.ceil(counts[:, :, r].max() / P)) for r in range(KR)]
    T = sum(T_r)
    t_off = np.concatenate([[0], np.cumsum(T_r)])  # tile offset per relation
    r_of_tile = sum(([r] * T_r[r] for r in range(KR)), [])

    # per-core meta arrays, layout [128(part), NBLK, T, field]
    idx_kv = np.zeros((NCORES, P, NBLK, T), dtype=np.int32)
    idx_qr = np.zeros((NCORES, P, NBLK, T), dtype=np.int32)
    dstl16 = np.full((NCORES, P, NBLK, T), DUMMY_DSTL, dtype=np.float16)

    sc, sb, sr = dst_core[eo], (dst_slot // P)[eo], edge_type[eo]
    # position of each edge within its (core, block, r) group
    grp = (sc * NBLK + sb) * KR + sr
    pos_in_grp = np.zeros(E, dtype=np.int64)
    _, first_idx, grp_cnt = np.unique(grp, return_index=True, return_counts=True)
    starts = np.zeros(E, dtype=np.int64)
    starts[first_idx] = np.arange(len(first_idx))
    pos_in_grp = np.arange(E) - np.repeat(first_idx, grp_cnt)

    slot = np.array(t_off, dtype=np.int64)[sr] * P + pos_in_grp  # within block
    tt, pp = slot // P, slot % P
    idx_kv[sc, pp, sb, tt] = src_new[eo].astype(np.int32)
    idx_qr[sc, pp, sb, tt] = (sr * Ncap + dst_slot[eo]).astype(np.int32)
    dstl16[sc, pp, sb, tt] = (dst_slot[eo] - sb * P).astype(np.float16)

    # field-major halves: [:, 0:NBLK*T] = kv indices, [:, NBLK*T:] = qrel indices
    meta_i32 = np.concatenate(
        [idx_kv.reshape(NCORES, P, NBLK * T), idx_qr.reshape(NCORES, P, NBLK * T)],
        axis=-1)
    meta_f16 = dstl16.reshape(NCORES, P, NBLK * T)

    # --- permuted node data ---
    x_perm = np.zeros((NCORES * Ncap, HID), dtype=np.float32)
    x_perm[valid] = x[perm[valid]]
    xT_full = np.ascontiguousarray(x_perm.T).astype(ml_dtypes.bfloat16)  # [128, 8*Ncap]
    xT_own = np.stack([
        np.ascontiguousarray(x_perm[c * Ncap:(c + 1) * Ncap].T)
        for c in range(NCORES)
    ]).astype(ml_dtypes.bfloat16)                                        # [8,128,Ncap]
    x_own = x_perm.reshape(NCORES, Ncap, HID)                            # f32

    # --- packed weights ---
    bf = ml_dtypes.bfloat16
    Wq = np.asarray(inputs["Wq"], np.float32)
    Wk = np.asarray(inputs["Wk"], np.float32)
    Wv = np.asarray(inputs["Wv"], np.float32)
    W_upd = np.asarray(inputs["W_upd"], np.float32)
    bq = np.asarray(inputs["bq"], np.float32)
    bk = np.asarray(inputs["bk"], np.float32)
    bv = np.asarray(inputs["bv"], np.float32)
    b_upd = np.asarray(inputs["b_upd"], np.float32)
    rel_pri = np.asarray(inputs["rel_pri"], np.float32)
    rel_att = np.asarray(inputs["rel_att"], np.float32)
    rel_msg = np.asarray(inputs["rel_msg"], np.float32)
    ln_gamma = np.asarray(inputs["ln_gamma"], np.float32)
    ln_beta = np.asarray(inputs["ln_beta"], np.float32)

    Wkv = np.concatenate([Wk, Wv], axis=-1).astype(bf)        # [KN,128,256]
    bkv = np.concatenate([bk, bv], axis=-1)[:, None, :].astype(bf)  # [KN,1,256]
    Wq_b = Wq.astype(bf)
    bq_col = bq[:, :, None].astype(np.float32)                # [KN,128,1]
    Bq = np.zeros((KR, HID, HID), dtype=np.float32)
    Bm = np.zeros((KR, HID, HID), dtype=np.float32)
    for r in range(KR):
        for h in range(H):
            s = slice(h * DK, (h + 1) * DK)
            Bq[r, s, s] = rel_att[r, h].T * (rel_pri[r, h] / SQRT_DK)
            Bm[r, s, s] = rel_msg[r, h]
    Bq16 = Bq.astype(np.float16)
    Bm16 = Bm.astype(np.float16)
    Wu16 = W_upd.astype(np.float16)
    bu_row = b_upd[:, None, :].astype(np.float16)             # [KN,1,128]
    gam_b = np.broadcast_to(ln_gamma[:, None, :], (KN, P, HID)).copy()
    bet_b = np.broadcast_to(ln_beta[:, None, :], (KN, P, HID)).copy()

    head_selT = np.zeros((H, HID), dtype=np.float16)
    for h in range(H):
        head_selT[h, h * DK:(h + 1) * DK] = 1.0
    iota128 = np.broadcast_to(np.arange(P, dtype=np.float16), (P, P)).copy()
    ones_row = np.ones((1, P), dtype=ml_dtypes.bfloat16)
    ones16 = np.ones((1, P), dtype=np.float16)

    shared = dict(
        xT_full=xT_full, Wkv=Wkv, bkv=bkv, Wq=Wq_b, bq_col=bq_col,
        Bq16=Bq16, Bm16=Bm16, Wu16=Wu16, bu_row=bu_row,
        gam_b=gam_b, bet_b=bet_b, head_selT=head_selT, iota128=iota128,
        ones_row=ones_row, ones16=ones16,
    )
    per_core = [dict(
        xT_own=np.ascontiguousarray(xT_own[c]),
        x_own=np.ascontiguousarray(x_own[c]),
        meta_i32=np.ascontiguousarray(meta_i32[c]),
        meta_f16=np.ascontiguousarray(meta_f16[c]),
    ) for c in range(NCORES)]

    geom = dict(Ncap=Ncap, NBLK=NBLK, K_t=K_t, cum_K=list(cum_K),
                T_r=T_r, T=T, t_off=list(t_off), r_of_tile=r_of_tile)
    return shared, per_core, geom, perm, valid


# ----------------------------------------------------------------------------
# Bass program
# ----------------------------------------------------------------------------

def _segments(geom):
    """(type, absolute node range) segments of the full permuted table."""
    Ncap, K_t, cum_K = geom["Ncap"], geom["K_t"], geom["cum_K"]
    segs = []
    for c in range(NCORES):
        for t in range(KN):
            a = c * Ncap + cum_K[t]
            segs.append((t, a, a + K_t[t]))
    return segs


def _own_segments(geom):
    K_t, cum_K = geom["K_t"], geom["cum_K"]
    return [(t, cum_K[t], cum_K[t] + K_t[t]) for t in range(KN)]


def _chunks(a, b, step):
    while a < b:
        yield a, min(a + step, b)
        a = min(a + step, b)


def build_program(geom, debug=False):
    Ncap, NBLK, T = geom["Ncap"], geom["NBLK"], geom["T"]
    T_r, t_off, r_of_tile = geom["T_r"], geom["t_off"], geom["r_of_tile"]
    internal_kind = "ExternalOutput" if debug else "Internal"

    nc = bacc.Bacc("TRN2", target_bir_lowering=False, debug=False,
                   enable_asserts=False, num_devices=NCORES)

    def inp(name, shape, dtype):
        return nc.dram_tensor(name, list(shape), dtype, kind="ExternalInput").ap()

    # shared inputs
    xT_full = inp("xT_full", [P, NCORES * Ncap], BF16)
    Wkv = inp("Wkv", [KN, P, 2 * HID], BF16)
    bkv = inp("bkv", [KN, 1, 2 * HID], BF16)
    Wq = inp("Wq", [KN, P, HID], BF16)
    bq_col = inp("bq_col", [KN, P, 1], F32)
    Bq16 = inp("Bq16", [KR, P, HID], F16)
    Bm16 = inp("Bm16", [KR, P, HID], F16)
    Wu16 = inp("Wu16", [KN, P, HID], F16)
    bu_row = inp("bu_row", [KN, 1, HID], F16)
    gam_b = inp("gam_b", [KN, P, HID], F32)
    bet_b = inp("bet_b", [KN, P, HID], F32)
    head_selT = inp("head_selT", [H, HID], F16)
    iota128 = inp("iota128", [P, P], F16)
    ones_row = inp("ones_row", [1, P], BF16)
    ones16 = inp("ones16", [1, P], F16)
    # per-core inputs
    xT_own = inp("xT_own", [P, Ncap], BF16)
    x_own = inp("x_own", [Ncap, HID], F32)
    meta_i32 = inp("meta_i32", [P, NBLK * T * 2], I32)
    meta_f16 = inp("meta_f16", [P, NBLK * T], F16)

    out_dram = nc.dram_tensor("out", [Ncap, HID], F32, kind="ExternalOutput").ap()
    kv_table = nc.dram_tensor("kv_table", [NCORES * Ncap, 2 * HID], F16,
                              kind=internal_kind).ap()
    qrel_table = nc.dram_tensor("qrel_table", [KR * Ncap, HID], F16,
                                kind=internal_kind).ap()
    aggrT_dram = nc.dram_tensor("aggrT_dram", [NBLK, P, P], F16,
                                kind=internal_kind).ap()
    if debug:
        dbg_kv = nc.dram_tensor("dbg_kv", [P, T * 2 * HID], F16,
                                kind="ExternalOutput").ap()
        dbg_qr = nc.dram_tensor("dbg_qr", [P, T * HID], F16,
                                kind="ExternalOutput").ap()
        dbg_oall = nc.dram_tensor("dbg_oall", [P, T * P], F16,
                                  kind="ExternalOutput").ap()
        dbg_att = nc.dram_tensor("dbg_att", [P, T * H], F32,
                                 kind="ExternalOutput").ap()
        dbg_wv = nc.dram_tensor("dbg_wv", [P, T * HID], F16,
                                kind="ExternalOutput").ap()
        dbg_den = nc.dram_tensor("dbg_den", [H, P], F32,
                                 kind="ExternalOutput").ap()
        dbg_par = nc.dram_tensor("dbg_par", [P, KR * P], F32,
                                 kind="ExternalOutput").ap()
        dbg_rexp = nc.dram_tensor("dbg_rexp", [P, P], F16,
                                  kind="ExternalOutput").ap()

    with tile.TileContext(nc) as tc:
        # ------- persistent small tiles -------
        with tc.tile_pool(name="const", bufs=1) as cpool:
            t_Wkv = [cpool.tile([P, 2 * HID], BF16, tag=f"wkv{t}", name=f"t_wkv{t}") for t in range(KN)]
            t_bkv = [cpool.tile([1, 2 * HID], BF16, tag=f"bkv{t}", name=f"t_bkv{t}") for t in range(KN)]
            t_Wq = [cpool.tile([P, HID], BF16, tag=f"wq{t}", name=f"t_wq{t}") for t in range(KN)]
            t_bq = [cpool.tile([P, 1], F32, tag=f"bq{t}", name=f"t_bq{t}") for t in range(KN)]
            t_Bq = [cpool.tile([P, HID], F16, tag=f"bqr{r}", name=f"t_bqr{r}") for r in range(KR)]
            t_Bm = [cpool.tile([P, HID], F16, tag=f"bmr{r}", name=f"t_bmr{r}") for r in range(KR)]
            t_Wu = [cpool.tile([P, HID], F16, tag=f"wu{t}", name=f"t_wu{t}") for t in range(KN)]
            t_bu = [cpool.tile([1, HID], F16, tag=f"bu{t}", name=f"t_bu{t}") for t in range(KN)]
            t_gam = [cpool.tile([P, HID], F32, tag=f"g{t}", name=f"t_g{t}") for t in range(KN)]
            t_bet = [cpool.tile([P, HID], F32, tag=f"b{t}", name=f"t_b{t}") for t in range(KN)]
            t_hsel = cpool.tile([H, HID], F16)
            t_iota = cpool.tile([P, P], F16)
            t_ones = cpool.tile([1, P], BF16)
            t_ones16 = cpool.tile([1, P], F16)
            t_meta_i = cpool.tile([P, NBLK * T * 2], I32)
            t_meta_f = cpool.tile([P, NBLK * T], F16)
            t_eps = cpool.tile([P, 1], F32)
            nc.vector.memset(t_eps[:], LN_EPS)

            for t in range(KN):
                nc.sync.dma_start(out=t_Wkv[t][:], in_=Wkv[t])
                nc.sync.dma_start(out=t_bkv[t][:], in_=bkv[t])
                nc.sync.dma_start(out=t_Wq[t][:], in_=Wq[t])
                nc.sync.dma_start(out=t_bq[t][:], in_=bq_col[t])
                nc.sync.dma_start(out=t_Wu[t][:], in_=Wu16[t])
                nc.sync.dma_start(out=t_bu[t][:], in_=bu_row[t])
                nc.sync.dma_start(out=t_gam[t][:], in_=gam_b[t])
                nc.sync.dma_start(out=t_bet[t][:], in_=bet_b[t])
            for r in range(KR):
                nc.sync.dma_start(out=t_Bq[r][:], in_=Bq16[r])
                nc.sync.dma_start(out=t_Bm[r][:], in_=Bm16[r])
            nc.sync.dma_start(out=t_hsel[:], in_=head_selT[:])
            nc.sync.dma_start(out=t_iota[:], in_=iota128[:])
            nc.sync.dma_start(out=t_ones[:], in_=ones_row[:])
            nc.sync.dma_start(out=t_ones16[:], in_=ones16[:])
            nc.sync.dma_start(out=t_meta_i[:], in_=meta_i32[:])
            nc.sync.dma_start(out=t_meta_f[:], in_=meta_f16[:])

            # ================= stage 1a: full kv table =================
            SLAB = 512
            with tc.tile_pool(name="s1", bufs=3) as pool, \
                 tc.tile_pool(name="s1p", bufs=2, space="PSUM") as ppool:
                for t, a, b in _segments(geom):
                    for c0, c1 in _chunks(a, b, SLAB):
                        xt = pool.tile([P, c1 - c0], BF16, tag="xt")
                        nc.sync.dma_start(out=xt[:], in_=xT_full[:, c0:c1])
                        kv_sb = pool.tile([P, (c1 - c0 + P - 1) // P * 2 * HID],
                                          F16, tag="kvsb")
                        for i, (d0, d1) in enumerate(_chunks(c0, c1, P)):
                            m = d1 - d0
                            ps = ppool.tile([P, 2 * HID], F32, tag="kvps")
                            nc.tensor.matmul(out=ps[:m, :], lhsT=xt[:, d0 - c0:d1 - c0],
                                             rhs=t_Wkv[t][:], start=True, stop=False)
                            nc.tensor.matmul(out=ps[:m, :], lhsT=t_ones[:, :m],
                                             rhs=t_bkv[t][:], start=False, stop=True)
                            nc.scalar.copy(out=kv_sb[:m, i * 2 * HID:(i + 1) * 2 * HID],
                                           in_=ps[:m, :])
                        nch = (c1 - c0 + P - 1) // P
                        kv_view = kv_sb[:].rearrange("p (c e) -> p c e", c=nch)
                        # DRAM rows c0..c1 viewed [chunk, p, e]
                        dr = kv_table[c0:c1, :].rearrange("(c p) e -> p c e", p=P) \
                            if (c1 - c0) % P == 0 else None
                        if dr is not None:
                            nc.scalar.dma_start(out=dr, in_=kv_view)
                        else:
                            for i, (d0, d1) in enumerate(_chunks(c0, c1, P)):
                                nc.scalar.dma_start(
                                    out=kv_table[d0:d1, :],
                                    in_=kv_sb[:d1 - d0, i * 2 * HID:(i + 1) * 2 * HID])

                # ================= stage 1b: own q / qrel tables =================
                for t, a, b in _own_segments(geom):
                    for c0, c1 in _chunks(a, b, SLAB):
                        w = c1 - c0
                        xo = pool.tile([P, SLAB], BF16, tag="xo")
                        nc.sync.dma_start(out=xo[:, :w], in_=xT_own[:, c0:c1])
                        qt_ps = ppool.tile([P, SLAB], F32, tag="qtps")
                        nc.tensor.matmul(out=qt_ps[:, :w], lhsT=t_Wq[t][:],
                                         rhs=xo[:, :w], start=True, stop=True)
                        qt_sb = pool.tile([P, SLAB], F16, tag="qtsb")
                        nc.scalar.activation(out=qt_sb[:, :w], in_=qt_ps[:, :w],
                                             func=mybir.ActivationFunctionType.Identity,
                                             bias=t_bq[t][:], scale=1.0)
                        for r in range(KR):
                            qr_sb = pool.tile([P, SLAB], F16, tag=f"qrsb{r}")
                            for i, (d0, d1) in enumerate(_chunks(c0, c1, P)):
                                m = d1 - d0
                                qr_ps = ppool.tile([P, HID], F32, tag="qrps")
                                nc.tensor.matmul(out=qr_ps[:m, :],
                                                 lhsT=qt_sb[:, d0 - c0:d1 - c0],
                                                 rhs=t_Bq[r][:], start=True, stop=True)
                                nc.scalar.copy(out=qr_sb[:m, i * HID:(i + 1) * HID],
                                               in_=qr_ps[:m, :])
                            if w % P == 0:
                                nch = w // P
                                nc.scalar.dma_start(
                                    out=qrel_table[r * Ncap + c0: r * Ncap + c1, :]
                                        .rearrange("(c p) e -> p c e", p=P),
                                    in_=qr_sb[:].rearrange("p (c e) -> p c e", c=nch))
                            else:
                                for i, (d0, d1) in enumerate(_chunks(c0, c1, P)):
                                    nc.scalar.dma_start(
                                        out=qrel_table[r * Ncap + d0: r * Ncap + d1, :],
                                        in_=qr_sb[:d1 - d0, i * HID:(i + 1) * HID])

            tc.strict_bb_all_engine_barrier()

            # ================= stage 2: edge phase =================
            with tc.tile_pool(name="s2", bufs=2) as pool, \
                 tc.tile_pool(name="s2p", bufs=2, space="PSUM") as ppool:
                for b in range(NBLK):
                    kv_blk = pool.tile([P, T * 2 * HID], F16, tag="kvblk")
                    qr_blk = pool.tile([P, T * HID], F16, tag="qrblk")
                    for t in range(T):
                        nc.gpsimd.indirect_dma_start(
                            out=kv_blk[:, t * 2 * HID:(t + 1) * 2 * HID],
                            out_offset=None, in_=kv_table[:],
                            in_offset=IndirectOffsetOnAxis(
                                ap=t_meta_i[:, b * T + t: b * T + t + 1], axis=0))
                        nc.gpsimd.indirect_dma_start(
                            out=qr_blk[:, t * HID:(t + 1) * HID],
                            out_offset=None, in_=qrel_table[:],
                            in_offset=IndirectOffsetOnAxis(
                                ap=t_meta_i[:, NBLK * T + b * T + t:
                                            NBLK * T + b * T + t + 1], axis=0))

                    dstl = t_meta_f[:, b * T: (b + 1) * T]
                    o_all = pool.tile([P, T * P], F16, tag="oall")
                    nc.vector.tensor_tensor(
                        out=o_all[:].rearrange("p (t j) -> p t j", t=T),
                        in0=t_iota[:].rearrange("p (a j) -> p a j", a=1)
                            .to_broadcast([P, T, P]),
                        in1=dstl.rearrange("p (t a) -> p t a", a=1)
                            .to_broadcast([P, T, P]),
                        op=mybir.AluOpType.is_equal)

                    prod = pool.tile([P, T * HID], F16, tag="prodb")
                    nc.vector.tensor_tensor(
                        out=prod[:],
                        in0=kv_blk[:].rearrange("p (t c) -> p t c", t=T)[:, :, 0:HID],
                        in1=qr_blk[:].rearrange("p (t c) -> p t c", t=T),
                        op=mybir.AluOpType.mult)
                    att = pool.tile([P, T * H], F32, tag="attb")
                    nc.vector.tensor_reduce(
                        out=att[:], in_=prod[:].rearrange("p (g d) -> p g d", d=DK),
                        axis=mybir.AxisListType.X, op=mybir.AluOpType.add)
                    expatt = pool.tile([P, T * H], F16, tag="expb")
                    nc.scalar.activation(out=expatt[:], in_=att[:],
                                         func=mybir.ActivationFunctionType.Exp)

                    # denominator: denomT[h, j] accumulated over tiles
                    denomT = ppool.tile([H, P], F32, tag="denps")
                    for t in range(T):
                        nc.tensor.matmul(out=denomT[:],
                                         lhsT=expatt[:, t * H:(t + 1) * H],
                                         rhs=o_all[:, t * P:(t + 1) * P],
                                         start=(t == 0), stop=(t == T - 1))

                    # unnormalized weighted values: wv = v * expatt (bcast over DK)
                    wv = pool.tile([P, T * HID], F16, tag="wvb")
                    nc.vector.tensor_tensor(
                        out=wv[:].rearrange("p (t h d) -> p t h d", h=H, d=DK),
                        in0=kv_blk[:].rearrange("p (t c) -> p t c", t=T)[:, :, HID:2 * HID]
                            .rearrange("p t (h d) -> p t h d", d=DK),
                        in1=expatt[:].rearrange("p (t h a) -> p t h a", h=H, a=1)
                            .to_broadcast([P, T, H, DK]),
                        op=mybir.AluOpType.mult)

                    # per-relation partials: partialT[f, r*128+j]
                    partialT = ppool.tile([P, KR * P], F32, tag="parps")
                    for t in range(T):
                        r = r_of_tile[t]
                        nc.tensor.matmul(out=partialT[:, r * P:(r + 1) * P],
                                         lhsT=wv[:, t * HID:(t + 1) * HID],
                                         rhs=o_all[:, t * P:(t + 1) * P],
                                         start=(t == t_off[r]),
                                         stop=(t == t_off[r + 1] - 1))

                    # softmax denominators -> reciprocal, expanded to feature rows
                    den_sb = pool.tile([H, P], F32, tag="densb")
                    nc.vector.tensor_scalar_add(out=den_sb[:], in0=denomT[:],
                                                scalar1=1e-16)
                    recip = pool.tile([H, P], F32, tag="recip")
                    nc.vector.reciprocal(out=recip[:], in_=den_sb[:])
                    recip16 = pool.tile([H, P], F16, tag="recip16")
                    nc.vector.tensor_scalar_min(out=recip16[:], in0=recip[:],
                                                scalar1=60000.0)
                    rexp_ps = ppool.tile([P, P], F32, tag="rexps")
                    nc.tensor.matmul(out=rexp_ps[:], lhsT=t_hsel[:],
                                     rhs=recip16[:], start=True, stop=True)
                    rexp = pool.tile([P, P], F16, tag="rexp")
                    nc.scalar.copy(out=rexp[:], in_=rexp_ps[:])

                    # normalize partials, apply Bmsg, accumulate aggrT
                    aggT_ps = ppool.tile([P, P], F32, tag="aggps")
                    for r in range(KR):
                        pT = pool.tile([P, P], F16, tag=f"pt{r}")
                        nc.vector.tensor_tensor(out=pT[:],
                                                in0=partialT[:, r * P:(r + 1) * P],
                                                in1=rexp[:],
                                                op=mybir.AluOpType.mult)
                        nc.tensor.matmul(out=aggT_ps[:], lhsT=t_Bm[r][:], rhs=pT[:],
                                         start=(r == 0), stop=(r == KR - 1))
                    aggT = pool.tile([P, P], F16, tag="aggsb")
                    nc.scalar.copy(out=aggT[:], in_=aggT_ps[:])
                    nc.sync.dma_start(out=aggrT_dram[b], in_=aggT[:])

                    if debug and b == 0:
                        nc.sync.dma_start(out=dbg_kv[:, :], in_=kv_blk[:])
                        nc.sync.dma_start(out=dbg_qr[:, :], in_=qr_blk[:])
                        nc.sync.dma_start(out=dbg_oall[:, :], in_=o_all[:])
                        nc.sync.dma_start(out=dbg_att[:, :], in_=att[:])
                        nc.sync.dma_start(out=dbg_wv[:, :], in_=wv[:])
                        nc.sync.dma_start(out=dbg_rexp[:, :], in_=rexp[:])
                        den_dump = pool.tile([H, P], F32, tag="dendmp")
                        nc.vector.tensor_copy(out=den_dump[:], in_=denomT[:])
                        nc.sync.dma_start(out=dbg_den[:, :], in_=den_dump[:])
                        par_dump = pool.tile([P, KR * P], F32, tag="pardmp")
                        nc.vector.tensor_copy(out=par_dump[:], in_=partialT[:])
                        nc.sync.dma_start(out=dbg_par[:, :], in_=par_dump[:])

            tc.strict_bb_all_engine_barrier()

            # ================= stage 3: update + layernorm =================
            with tc.tile_pool(name="s3", bufs=3) as pool, \
                 tc.tile_pool(name="s3p", bufs=2, space="PSUM") as ppool:
                for t, a, b in _own_segments(geom):
                    for d0, d1 in _chunks(a, b, P):
                        m = d1 - d0
                        blk, off = d0 // P, d0 % P
                        gaT = pool.tile([P, P], F16, tag="gaT")
                        # aggrT block columns off..off+m hold nodes d0..d1
                        # (only when chunk is block-aligned; segments are not
                        # 128-aligned so handle the general span)
                        if off + m <= P:
                            nc.sync.dma_start(out=gaT[:, :m],
                                              in_=aggrT_dram[blk, :, off:off + m])
                        else:
                            m1 = P - off
                            nc.sync.dma_start(out=gaT[:, :m1],
                                              in_=aggrT_dram[blk, :, off:P])
                            nc.sync.dma_start(out=gaT[:, m1:m],
                                              in_=aggrT_dram[blk + 1, :, 0:m - m1])
                        ga2 = pool.tile([P, P], F16, tag="ga2")
                        nc.scalar.activation(out=ga2[:, :m], in_=gaT[:, :m],
                                             func=mybir.ActivationFunctionType.Gelu)
                        h_ps = ppool.tile([P, HID], F32, tag="hps")
                        nc.tensor.matmul(out=h_ps[:m, :], lhsT=ga2[:, :m],
                                         rhs=t_Wu[t][:], start=True, stop=False)
                        nc.tensor.matmul(out=h_ps[:m, :], lhsT=t_ones16[:, :m],
                                         rhs=t_bu[t][:], start=False, stop=True)
                        xc = pool.tile([P, HID], F32, tag="xc")
                        nc.sync.dma_start(out=xc[:m, :], in_=x_own[d0:d1, :])
                        s = pool.tile([P, HID], F32, tag="ssum")
                        nc.vector.tensor_tensor(out=s[:m, :], in0=h_ps[:m, :],
                                                in1=xc[:m, :], op=mybir.AluOpType.add)
                        mean = pool.tile([P, 1], F32, tag="mean")
                        nc.vector.tensor_reduce(out=mean[:m], in_=s[:m, :],
                                                axis=mybir.AxisListType.X,
                                                op=mybir.AluOpType.add)
                        nc.scalar.mul(out=mean[:m], in_=mean[:m], mul=1.0 / HID)
                        nc.vector.tensor_scalar_sub(out=s[:m, :], in0=s[:m, :],
                                                    scalar1=mean[:m])
                        sq = pool.tile([P, HID], F32, tag="sq")
                        var = pool.tile([P, 1], F32, tag="var")
                        nc.scalar.activation(out=sq[:m, :], in_=s[:m, :],
                                             func=mybir.ActivationFunctionType.Square,
                                             accum_out=var[:m])
                        std = pool.tile([P, 1], F32, tag="std")
                        nc.scalar.activation(out=std[:m], in_=var[:m],
                                             func=mybir.ActivationFunctionType.Sqrt,
                                             bias=t_eps[:m], scale=1.0 / HID)
                        nc.vector.reciprocal(out=std[:m], in_=std[:m])
                        nc.vector.tensor_scalar_mul(out=s[:m, :], in0=s[:m, :],
                                                    scalar1=std[:m])
                        o = pool.tile([P, HID], F32, tag="obuf")
                        nc.vector.tensor_tensor(out=o[:m, :], in0=s[:m, :],
                                                in1=t_gam[t][:m, :],
                                                op=mybir.AluOpType.mult)
                        nc.vector.tensor_tensor(out=o[:m, :], in0=o[:m, :],
                                                in1=t_bet[t][:m, :],
                                                op=mybir.AluOpType.add)
                        nc.sync.dma_start(out=out_dram[d0:d1, :], in_=o[:m, :])

    nc.compile()
    return nc


# ----------------------------------------------------------------------------
# Entry point
# ----------------------------------------------------------------------------

_CACHE = {}
_LAST_RESULT = None


def kernel(**inputs):
    global _LAST_RESULT
    shared, per_core, geom, perm, valid = _prep(inputs)
    key = (geom["Ncap"], tuple(geom["T_r"]), tuple(geom["K_t"]))
    if key not in _CACHE:
        _CACHE[key] = build_program(geom)
    nc = _CACHE[key]

    in_maps = []
    for c in range(NCORES):
        m = dict(shared)
        m.update(per_core[c])
        in_maps.append({k: np.asarray(v) for k, v in m.items()})
    res = run_bass_kernel_spmd(nc, in_maps, list(range(NCORES)))
    _LAST_RESULT = res

    Ncap = geom["Ncap"]
    out_perm = np.concatenate([res.results[c]["out"] for c in range(NCORES)], axis=0)
    out = np.zeros((N, HID), dtype=np.float32)
    out[perm[valid]] = out_perm[valid]
    return out
